# revision 13
# baseline (speedup 1.0000x reference)
"""TRN2 Bass kernel for ConvNeXt-MLP + parallel top-2-of-3 LoRA-MoE.

Data-parallel over the token dim across 8 NeuronCores (12544 tokens ->
1568/core). All weights replicated. Per core, everything is computed in
feature-major ("transposed") layout: activations live in SBUF as
[features_on_partitions, tokens_on_free_dim]; the host transposes x in and
the output back out.

All matmuls run in bf16 (1 cycle/row on the PE, and the 2-byte LDWEIGHTS
hides under the matmul, unlike the 4-byte f32r weight load). w1 and w2 are
fully SBUF-resident in bf16 (9.4 MB), so the hidden dim is processed in a
single 24-chunk pass per token tile with the output accumulating entirely
in PSUM - no SBUF accumulation round-trips.

Math per core (T = 1568 tokens, tiles of 448/448/448/224):
  phase A: [logits | lora_down]^T = [rw | wd]^T @ xT  (merged bf16 matmul)
           acts = gelu(lora_down)
  softmax: logits transposed to token-major via one DVE 32x32 stream
           transpose; softmax + top-2-of-3 + renormalize fully batched
           (free-dim broadcast APs); comb transposed back the same way.
  phase B (per tile): h_j = gelu(w1_j^T @ xT + b1_j) for j in 0..23,
           out accumulates w2_j^T @ h_j in 6 PSUM banks across all j,
           plus wu^T @ (acts * expand(comb)), then + b2 -> DMA out.
"""

import numpy as np
import ml_dtypes

import concourse.bacc as bacc
import concourse.mybir as mybir
import concourse.tile as tile
from concourse.bass_utils import run_bass_kernel_spmd

F32 = mybir.dt.float32
BF16 = mybir.dt.bfloat16
AF = mybir.ActivationFunctionType
ALU = mybir.AluOpType
AX = mybir.AxisListType

NCORES = 8
B, N, D = 64, 196, 768
T = B * N                  # 12544 tokens total
TC = T // NCORES           # 1568 tokens per core
HID = 4 * D                # 3072
E, R = 3, 8
ER = E * R                 # 24
DC = D // 128              # 6 input-feature chunks
HC = HID // 128            # 24 hidden chunks
MC = D // 128              # 6 output chunks
NT_SIZES = [448, 448, 448, 224]    # token tiles per core (sum = 1568)
NBLK = TC // 32            # 49 32-token blocks for the stream transpose

_cache = {}


def _build():
    nc = bacc.Bacc("TRN2", target_bir_lowering=False, debug=False)

    xt_d = nc.dram_tensor("xt", [D, TC], BF16, kind="ExternalInput")
    xlo_d = nc.dram_tensor("xlo", [D, TC], BF16, kind="ExternalInput")
    w1_d = nc.dram_tensor("w1", [D, HID], BF16, kind="ExternalInput")
    w2_d = nc.dram_tensor("w2", [HID, D], BF16, kind="ExternalInput")
    wu_d = nc.dram_tensor("wu", [ER, D], BF16, kind="ExternalInput")
    b1_d = nc.dram_tensor("b1r", [128, HC], F32, kind="ExternalInput")
    b2_d = nc.dram_tensor("b2r", [128, MC], F32, kind="ExternalInput")
    rwd_d = nc.dram_tensor("rwd", [D, 96], BF16, kind="ExternalInput")
    rb_d = nc.dram_tensor("rb", [E, 1], F32, kind="ExternalInput")
    bx_d = nc.dram_tensor("bexp", [E, ER], BF16, kind="ExternalInput")
    out_d = nc.dram_tensor("outT", [D, TC], F32, kind="ExternalOutput")

    with tile.TileContext(nc) as tc:
        with (
            tc.tile_pool(name="const", bufs=1) as cp,
            tc.tile_pool(name="big", bufs=1) as bp,
            tc.tile_pool(name="hbuf", bufs=3) as hp,
            tc.tile_pool(name="osb", bufs=2) as op,
        ):
            # ---- resident loads ----
            # gpsimd DMA queue: xt tile 0 + router consts first (they gate the
            # first PE matmul), then x_lo tiles + remaining consts. The sync
            # queue streams xt tiles 1-3 and w1; w2 rides the vector queue.
            xts, xlos = [None] * 4, [None] * 4
            xt_view = xt_d.rearrange("(c p) t -> p c t", p=128)
            xlo_view = xlo_d.rearrange("(c p) t -> p c t", p=128)
            xts[0] = bp.tile([128, DC * NT_SIZES[0]], BF16, tag="xt0",
                             name="xt0")
            nc.gpsimd.dma_start(
                xts[0][:].rearrange("p (c t) -> p c t", c=DC),
                xt_view[:, :, 0:NT_SIZES[0]],
            )
            rwd = cp.tile([128, DC * 96], BF16, tag="rwd")
            nc.gpsimd.dma_start(
                rwd[:].rearrange("p (c e) -> p c e", c=DC),
                rwd_d.rearrange("(c p) e -> p c e", p=128),
            )
            xlos[0] = bp.tile([128, DC * NT_SIZES[0]], BF16, tag="xlo0",
                              name="xlo0")
            nc.gpsimd.dma_start(
                xlos[0][:].rearrange("p (c t) -> p c t", c=DC),
                xlo_view[:, :, 0:NT_SIZES[0]],
            )
            wu = cp.tile([ER, D], BF16, tag="wu")
            nc.gpsimd.dma_start(wu[:], wu_d[:])
            b1 = cp.tile([128, HC], F32, tag="b1")
            nc.gpsimd.dma_start(b1[:], b1_d[:])
            b2 = cp.tile([128, MC], F32, tag="b2")
            nc.gpsimd.dma_start(b2[:], b2_d[:])
            rb = cp.tile([E, 1], F32, tag="rb")
            nc.gpsimd.dma_start(rb[:], rb_d[:])
            bx = cp.tile([E, ER], BF16, tag="bx")
            nc.gpsimd.dma_start(bx[:], bx_d[:])

            # xt tiles 1-3 on sync, x_lo tiles 1-3 on gpsimd
            t0 = NT_SIZES[0]
            for i, n in enumerate(NT_SIZES):
                if i == 0:
                    continue
                x_i = bp.tile([128, DC * n], BF16, tag=f"xt{i}", name=f"xt{i}")
                nc.sync.dma_start(
                    x_i[:].rearrange("p (c t) -> p c t", c=DC),
                    xt_view[:, :, t0:t0 + n],
                )
                xts[i] = x_i
                xl_i = bp.tile([128, DC * n], BF16, tag=f"xlo{i}",
                               name=f"xlo{i}")
                nc.gpsimd.dma_start(
                    xl_i[:].rearrange("p (c t) -> p c t", c=DC),
                    xlo_view[:, :, t0:t0 + n],
                )
                xlos[i] = xl_i
                t0 += n

            # w1 [128, c, hid] on sync / w2 [128, j, d] on vector, resident
            # bf16, loaded in 4 chunks each so early j's are ready early
            w1s = bp.tile([128, DC * HID], BF16, tag="w1s")
            w2s = bp.tile([128, HC * D], BF16, tag="w2s")
            w1v = w1s[:].rearrange("p (c f) -> p c f", c=DC)
            w2v = w2s[:].rearrange("p (j f) -> p j f", j=HC)
            for q in range(4):
                h0, h1 = q * (HID // 4), (q + 1) * (HID // 4)
                nc.sync.dma_start(
                    w1v[:, :, h0:h1],
                    w1_d.rearrange("(c p) f -> p c f", p=128)[:, :, h0:h1],
                )
                j0, j1 = q * (HC // 4), (q + 1) * (HC // 4)
                nc.scalar.dma_start(
                    w2v[:, j0:j1, :],
                    w2_d.rearrange("(j p) f -> p j f", p=128)[:, j0:j1, :],
                )

            lgT = bp.tile([32, TC], F32, tag="lgT")
            nc.vector.memset(lgT[:], 0.0)
            acts = bp.tile([ER, TC], F32, tag="acts")
            scaled = bp.tile([ER, TC], BF16, tag="scaled")

            # ---- phase A: router logits + LoRA expert activations ----
            # merged [rw_hi(0:3) | rw_lo(3:6) | pad | 24 lora-down] @ x_hi,
            # then the compensation chain x_lo @ [rw_hi | rw_lo] accumulated
            # into the same PSUM columns. logits = dn[0:3] + dn[3:6] + rb is
            # then bf16-pair exact (~6e-6), so the top-2 selection matches the
            # f32 reference and no expert flips occur.
            with tc.tile_pool(name="psA", bufs=1, space="PSUM") as psA:
                t0 = 0
                for i, n in enumerate(NT_SIZES):
                    dn27 = psA.tile([88, 512], F32, tag="dn27", bufs=2,
                                    name=f"dn27_{t0}")
                    for c in range(DC):
                        nc.tensor.matmul(
                            dn27[:, :n],
                            rwd[:, c * 96:c * 96 + 88],
                            xts[i][:, c * n:(c + 1) * n],
                            start=(c == 0), stop=(c == DC - 1),
                        )
                    for c in range(DC):
                        nc.tensor.matmul(
                            dn27[:3, :n],
                            rwd[:, c * 96:c * 96 + 3],
                            xlos[i][:, c * n:(c + 1) * n],
                            start=False, stop=(c == DC - 1),
                            skip_group_check=True,
                        )
                    nc.vector.tensor_scalar_add(lgT[:E, t0:t0 + n],
                                                dn27[:E, :n], rb[:])
                    nc.vector.tensor_add(lgT[:E, t0:t0 + n],
                                         lgT[:E, t0:t0 + n],
                                         dn27[32:32 + E, :n])
                    nc.scalar.activation(acts[:, t0:t0 + n], dn27[64:, :n],
                                         AF.Gelu)
                    t0 += n

            # token-major logits via one DVE 32x32 stream transpose:
            # lgtok[p, b*32+e] = logits[e, b*32+p]
            lgtok = bp.tile([32, TC], F32, tag="lgtok")
            nc.vector.transpose(lgtok[:], lgT[:])
            ltv = lgtok[:].rearrange("p (b q) -> p b q", b=NBLK)[:, :, :E]
            probs = bp.tile([32, NBLK * E], F32, tag="probs")
            prv = probs[:].rearrange("p (b q) -> p b q", b=NBLK)
            nc.scalar.activation(prv, ltv, AF.Exp)

            # batched softmax + top-2-of-3 renormalized combine weights:
            # comb_e = (p_e > p_min) * p_e / (sum - min + eps')
            ssum = bp.tile([32, NBLK], F32, tag="ssum")
            nc.vector.tensor_reduce(ssum[:], prv, axis=AX.X, op=ALU.add)
            pmin = bp.tile([32, NBLK], F32, tag="pmin")
            nc.vector.tensor_reduce(pmin[:], prv, axis=AX.X, op=ALU.min)
            rs = bp.tile([32, NBLK], F32, tag="rs")
            nc.vector.reciprocal(rs[:], ssum[:])
            den = bp.tile([32, NBLK], F32, tag="den")
            # den = (ssum - pmin) * rs + 1e-6
            nc.vector.tensor_sub(den[:], ssum[:], pmin[:])
            nc.vector.tensor_mul(den[:], den[:], rs[:])
            nc.vector.tensor_scalar_add(den[:], den[:], 1e-6)
            invd = bp.tile([32, NBLK], F32, tag="invd")
            nc.vector.reciprocal(invd[:], den[:])
            t1 = bp.tile([32, NBLK], F32, tag="t1")
            nc.vector.tensor_mul(t1[:], rs[:], invd[:])
            combt = bp.tile([32, NBLK * 32], BF16, tag="combt")
            cbv = combt[:].rearrange("p (b q) -> p b q", b=NBLK)[:, :, :E]
            mask = bp.tile([32, NBLK * E], F32, tag="mask")
            mkv = mask[:].rearrange("p (b q) -> p b q", b=NBLK)
            pminb = pmin[:].unsqueeze(2).broadcast_to([32, NBLK, E])
            nc.vector.tensor_tensor(mkv, prv, pminb, op=ALU.is_gt)
            nc.vector.tensor_mul(mkv, mkv, prv)
            t1b = t1[:].unsqueeze(2).broadcast_to([32, NBLK, E])
            nc.vector.tensor_tensor(cbv, mkv, t1b, op=ALU.mult)
            # back to expert-major [3, T] (rows 3..31 are garbage, unread)
            combT = bp.tile([32, TC], BF16, tag="combT")
            nc.vector.transpose(combT[:], combt[:])

            # ---- phase B: base MLP with single-pass PSUM accumulation ----
            with (
                tc.tile_pool(name="psO", bufs=1, space="PSUM") as psO,
                tc.tile_pool(name="psH", bufs=2, space="PSUM") as psH,
            ):
                t0 = 0
                for nt, n in enumerate(NT_SIZES):
                    outp = [psO.tile([128, 512], F32, tag=f"out{m}",
                                     name=f"out{m}_{nt}")
                            for m in range(MC)]
                    hsb_prev = None
                    for j in range(HC + 1):
                        if j < HC:
                            hps = psH.tile([128, 512], F32, tag="h",
                                           name=f"h_{nt}_{j}")
                            for c in range(DC):
                                nc.tensor.matmul(
                                    hps[:, :n],
                                    w1v[:, c, j * 128:(j + 1) * 128],
                                    xts[nt][:, c * n:(c + 1) * n],
                                    start=(c == 0), stop=(c == DC - 1),
                                )
                            hsb = hp.tile([128, 512], BF16, tag="hs",
                                          name=f"hs_{nt}_{j}")
                            nc.scalar.activation(
                                hsb[:, :n], hps[:, :n], AF.Gelu,
                                bias=b1[:, j:j + 1],
                            )
                        if j >= 1:
                            jj = j - 1
                            for m in range(MC):
                                nc.tensor.matmul(
                                    outp[m][:, :n],
                                    w2v[:, jj, m * 128:(m + 1) * 128],
                                    hsb_prev[:, :n],
                                    start=(jj == 0), stop=False,
                                )
                        hsb_prev = hsb
                        # mid-loop of tile 0: expand comb to the 24 LoRA rows
                        # and scale acts; comb is ready by now and the PE pays
                        # ~0.7us here instead of stalling at the first LoRA-up
                        if nt == 0 and j == 12:
                            tq = 0
                            for i2, n2 in enumerate(NT_SIZES):
                                ex = psH.tile([ER, 512], F32, tag="h",
                                              name=f"ex_{tq}")
                                nc.tensor.matmul(ex[:, :n2], bx[:],
                                                 combT[:E, tq:tq + n2],
                                                 start=True, stop=True)
                                nc.vector.tensor_mul(scaled[:, tq:tq + n2],
                                                     acts[:, tq:tq + n2],
                                                     ex[:, :n2])
                                tq += n2
                    # LoRA-up closes the PSUM accumulation group
                    for m in range(MC):
                        nc.tensor.matmul(
                            outp[m][:, :n],
                            wu[:, m * 128:(m + 1) * 128],
                            scaled[:, t0:t0 + n],
                            start=False, stop=True,
                        )
                    # bias-add + PSUM evacuate; the last tile alternates
                    # Scalar/DVE and splits the store so the tail is short
                    last = nt == len(NT_SIZES) - 1
                    osb = op.tile([128, MC * 512], F32, tag="osb",
                                  name=f"osb_{nt}")
                    for m in range(MC):
                        if last and m % 2 == 1:
                            nc.vector.tensor_scalar_add(
                                osb[:, m * 512:m * 512 + n], outp[m][:, :n],
                                b2[:, m:m + 1],
                            )
                        else:
                            nc.scalar.activation(
                                osb[:, m * 512:m * 512 + n], outp[m][:, :n],
                                AF.Identity, bias=b2[:, m:m + 1],
                            )
                    odv = out_d.rearrange("(m p) t -> p m t", p=128)
                    osv = osb[:].rearrange("p (m t) -> p m t", m=MC)
                    if last:
                        nc.sync.dma_start(odv[:, :MC // 2, t0:t0 + n],
                                          osv[:, :MC // 2, :n])
                        nc.sync.dma_start(odv[:, MC // 2:, t0:t0 + n],
                                          osv[:, MC // 2:, :n])
                    else:
                        nc.sync.dma_start(odv[:, :, t0:t0 + n],
                                          osv[:, :, :n])
                    t0 += n

    nc.compile()
    return nc


def _pack_rwd(router_w, w_down):
    rw = np.asarray(router_w, np.float32)
    rw_hi = rw.astype(ml_dtypes.bfloat16).astype(np.float32)
    rwd = np.zeros((D, 96), ml_dtypes.bfloat16)
    rwd[:, :E] = rw_hi
    rwd[:, 32:32 + E] = rw - rw_hi
    rwd[:, 64:88] = np.asarray(w_down, np.float32).transpose(1, 0, 2).reshape(D, ER)
    return rwd


def _bf16(a):
    return np.ascontiguousarray(
        np.asarray(a, np.float32).astype(ml_dtypes.bfloat16))


def _prep_inputs(x, w1, b1, w2, b2, router_w, router_b, w_down, w_up):
    x = np.asarray(x, dtype=np.float32)
    xT = x.reshape(T, D).T  # [D, T]
    common = {
        "w1": _bf16(w1),
        "w2": _bf16(w2),
        "wu": _bf16(np.asarray(w_up, np.float32).reshape(ER, D)),
        "b1r": np.ascontiguousarray(
            np.asarray(b1, np.float32).reshape(HC, 128).T),
        "b2r": np.ascontiguousarray(
            np.asarray(b2, np.float32).reshape(MC, 128).T),
        "rwd": np.ascontiguousarray(_pack_rwd(router_w, w_down)),
        "rb": np.ascontiguousarray(
            np.asarray(router_b, np.float32).reshape(E, 1)),
        "bexp": _bf16(np.repeat(np.eye(E, dtype=np.float32), R, axis=1)),
    }
    xT_hi = xT.astype(ml_dtypes.bfloat16)
    xT_lo = (xT - xT_hi.astype(np.float32)).astype(ml_dtypes.bfloat16)
    in_maps = []
    for c in range(NCORES):
        m = dict(common)
        m["xt"] = np.ascontiguousarray(xT_hi[:, c * TC:(c + 1) * TC])
        m["xlo"] = np.ascontiguousarray(xT_lo[:, c * TC:(c + 1) * TC])
        in_maps.append(m)
    return in_maps


def _run(inputs, trace=False):
    if "nc" not in _cache:
        _cache["nc"] = _build()
    nc = _cache["nc"]
    in_maps = _prep_inputs(**inputs)
    res = run_bass_kernel_spmd(nc, in_maps, core_ids=list(range(NCORES)),
                               trace=trace)
    outT = np.concatenate([res.results[c]["outT"] for c in range(NCORES)],
                          axis=1)  # [D, T]
    out = np.ascontiguousarray(outT.T).reshape(B, N, D).astype(np.float32)
    return out, res


def kernel(**inputs):
    return _run(inputs)[0]


# revision 17
# speedup vs baseline: 1.1044x; 1.1044x over previous
"""TRN2 Bass kernel for ConvNeXt-MLP + parallel top-2-of-3 LoRA-MoE.

Data-parallel over the token dim across 8 NeuronCores (12544 tokens ->
1568/core). All weights replicated. Per core, everything is computed in
feature-major ("transposed") layout: activations live in SBUF as
[features_on_partitions, tokens_on_free_dim]; the host transposes x in and
the output back out.

All matmuls run in bf16 (1 cycle/row on the PE; the 2-byte LDWEIGHTS hides
under the matmul, unlike the 4-byte f32r weight load). w1 and w2 are fully
SBUF-resident in bf16 (9.4 MB), so the hidden dim is a single 24-chunk pass
per token tile with the output accumulating entirely in PSUM.

Router exactness: bf16 alone flips the top-2 selection on ~8 near-tie
tokens (each flip is a ~0.1 abs output error). The kernel therefore
computes logits as x_hi@rw_hi + x_hi@rw_lo + x_lo@rw_hi with bf16 hi/lo
splits of both operands (max logit err ~2e-5 vs the f32 reference, smallest
top-2 margin in-distribution is 5.3e-5 -> selection is bit-identical).
rw_lo/lora-down live at PSUM quadrant offsets 32/64 (engine partition-base
rule). Softmax + top-2 + renormalize run fully batched on DVE via one
32x32 stream transpose each way.

Scheduling: one need-ordered DMA stream on the sync queue (xt0, rwd, xlo0,
then w1/w2 j-chunks interleaved with the remaining xt/xlo tiles in exact
consumption order - the queues fair-share HBM bandwidth, so priority =
order). Phase A for tiles 1-3 is interleaved INTO tile 0's j-loop (at
j=4/8/12) so their x DMAs arrive under compute. PSUM->SBUF output copies
run on the otherwise-idle Vector engine, interleaved with the LoRA-up
matmuls per output chunk.
"""

import numpy as np
import ml_dtypes

import concourse.bacc as bacc
import concourse.mybir as mybir
import concourse.tile as tile
from concourse.bass_utils import run_bass_kernel_spmd

F32 = mybir.dt.float32
BF16 = mybir.dt.bfloat16
AF = mybir.ActivationFunctionType
ALU = mybir.AluOpType
AX = mybir.AxisListType

NCORES = 8
B, N, D = 64, 196, 768
T = B * N                  # 12544 tokens total
TC = T // NCORES           # 1568 tokens per core
HID = 4 * D                # 3072
E, R = 3, 8
ER = E * R                 # 24
DC = D // 128              # 6 input-feature chunks
HC = HID // 128            # 24 hidden chunks
MC = D // 128              # 6 output chunks
NT_SIZES = [448, 448, 448, 224]    # token tiles per core (sum = 1568)
NT_OFF = [0, 448, 896, 1344]
NBLK = TC // 32            # 49 32-token blocks for the stream transpose
# w1/w2 are DMA'd in j-chunks in consumption order, interleaved with the
# xt/xlo tiles of later token tiles
WCHUNKS = [(0, 4), (4, 8), (8, 12), (12, 17), (17, 24)]

_cache = {}


def _build():
    nc = bacc.Bacc("TRN2", target_bir_lowering=False, debug=False)

    xt_d = nc.dram_tensor("xt", [D, TC], BF16, kind="ExternalInput")
    xlo_d = nc.dram_tensor("xlo", [D, TC], BF16, kind="ExternalInput")
    w1_d = nc.dram_tensor("w1", [D, HID], BF16, kind="ExternalInput")
    w2_d = nc.dram_tensor("w2", [HID, D], BF16, kind="ExternalInput")
    wu_d = nc.dram_tensor("wu", [ER, D], BF16, kind="ExternalInput")
    b1_d = nc.dram_tensor("b1r", [128, HC], F32, kind="ExternalInput")
    b2_d = nc.dram_tensor("b2r", [128, MC], F32, kind="ExternalInput")
    rwd_d = nc.dram_tensor("rwd", [D, 96], BF16, kind="ExternalInput")
    rb_d = nc.dram_tensor("rb", [E, 1], F32, kind="ExternalInput")
    bx_d = nc.dram_tensor("bexp", [E, ER], BF16, kind="ExternalInput")
    out_d = nc.dram_tensor("outT", [D, TC], F32, kind="ExternalOutput")

    with tile.TileContext(nc) as tc:
        with (
            tc.tile_pool(name="const", bufs=1) as cp,
            tc.tile_pool(name="big", bufs=1) as bp,
            tc.tile_pool(name="hbuf", bufs=3) as hp,
            tc.tile_pool(name="osb", bufs=2) as op,
            tc.tile_pool(name="psO", bufs=1, space="PSUM") as psO,
            tc.tile_pool(name="psH", bufs=2, space="PSUM") as psH,
        ):
            # ---- tiles ----
            xts = [bp.tile([128, DC * n], BF16, tag=f"xt{i}", name=f"xt{i}")
                   for i, n in enumerate(NT_SIZES)]
            xlos = [bp.tile([128, DC * n], BF16, tag=f"xlo{i}",
                            name=f"xlo{i}")
                    for i, n in enumerate(NT_SIZES)]
            rwd = cp.tile([128, DC * 96], BF16, tag="rwd")
            wu = cp.tile([ER, D], BF16, tag="wu")
            b1 = cp.tile([128, HC], F32, tag="b1")
            b2 = cp.tile([128, MC], F32, tag="b2")
            rb = cp.tile([E, 1], F32, tag="rb")
            bx = cp.tile([E, ER], BF16, tag="bx")
            w1s = bp.tile([128, DC * HID], BF16, tag="w1s")
            w2s = bp.tile([128, HC * D], BF16, tag="w2s")
            w1v = w1s[:].rearrange("p (c f) -> p c f", c=DC)
            w2v = w2s[:].rearrange("p (j f) -> p j f", j=HC)
            lgT = bp.tile([32, TC], F32, tag="lgT")
            acts = bp.tile([ER, TC], F32, tag="acts")
            scaled = bp.tile([ER, TC], BF16, tag="scaled")

            xt_view = xt_d.rearrange("(c p) t -> p c t", p=128)
            xlo_view = xlo_d.rearrange("(c p) t -> p c t", p=128)
            w1_view = w1_d.rearrange("(c p) f -> p c f", p=128)
            w2_view = w2_d.rearrange("(j p) f -> p j f", p=128)

            def load_x(i):
                lo, hi = NT_OFF[i], NT_OFF[i] + NT_SIZES[i]
                nc.sync.dma_start(
                    xts[i][:].rearrange("p (c t) -> p c t", c=DC),
                    xt_view[:, :, lo:hi])
                nc.sync.dma_start(
                    xlos[i][:].rearrange("p (c t) -> p c t", c=DC),
                    xlo_view[:, :, lo:hi])

            def load_w(q):
                j0, j1 = WCHUNKS[q]
                nc.sync.dma_start(w1v[:, :, j0 * 128:j1 * 128],
                                  w1_view[:, :, j0 * 128:j1 * 128])
                nc.sync.dma_start(w2v[:, j0:j1, :], w2_view[:, j0:j1, :])

            # need-ordered single DMA stream (sync); tiny consts on gpsimd
            nc.sync.dma_start(
                xts[0][:].rearrange("p (c t) -> p c t", c=DC),
                xt_view[:, :, 0:NT_SIZES[0]])
            nc.sync.dma_start(
                rwd[:].rearrange("p (c e) -> p c e", c=DC),
                rwd_d.rearrange("(c p) e -> p c e", p=128))
            nc.sync.dma_start(
                xlos[0][:].rearrange("p (c t) -> p c t", c=DC),
                xlo_view[:, :, 0:NT_SIZES[0]])
            nc.gpsimd.dma_start(rb[:], rb_d[:])
            nc.gpsimd.dma_start(b1[:], b1_d[:])
            nc.gpsimd.dma_start(b2[:], b2_d[:])
            nc.gpsimd.dma_start(bx[:], bx_d[:])
            nc.gpsimd.dma_start(wu[:], wu_d[:])
            load_w(0)
            load_x(1)
            load_w(1)
            load_x(2)
            load_w(2)
            load_x(3)
            load_w(3)
            load_w(4)

            nc.vector.memset(lgT[:], 0.0)

            # ---- phase A (per tile): router logits + LoRA activations ----
            def phase_a(i):
                n, t0 = NT_SIZES[i], NT_OFF[i]
                dn27 = psH.tile([88, 512], F32, tag="h", name=f"dn27_{i}")
                for c in range(DC):
                    nc.tensor.matmul(
                        dn27[:, :n],
                        rwd[:, c * 96:c * 96 + 88],
                        xts[i][:, c * n:(c + 1) * n],
                        start=(c == 0), stop=(c == DC - 1),
                    )
                for c in range(DC):
                    nc.tensor.matmul(
                        dn27[:3, :n],
                        rwd[:, c * 96:c * 96 + 3],
                        xlos[i][:, c * n:(c + 1) * n],
                        start=False, stop=(c == DC - 1),
                        skip_group_check=True,
                    )
                nc.vector.tensor_scalar_add(lgT[:E, t0:t0 + n],
                                            dn27[:E, :n], rb[:])
                nc.vector.tensor_add(lgT[:E, t0:t0 + n],
                                     lgT[:E, t0:t0 + n],
                                     dn27[32:32 + E, :n])
                nc.scalar.activation(acts[:, t0:t0 + n], dn27[64:, :n],
                                     AF.Gelu)

            # batched softmax + top-2-of-3 renormalized combine weights:
            # comb_e = (p_e > p_min) * p_e / ((sum - min)/sum + 1e-6) / sum
            def softmax_block():
                lgtok = bp.tile([32, TC], F32, tag="lgtok")
                nc.vector.transpose(lgtok[:], lgT[:])
                ltv = lgtok[:].rearrange("p (b q) -> p b q", b=NBLK)[:, :, :E]
                probs = bp.tile([32, NBLK * E], F32, tag="probs")
                prv = probs[:].rearrange("p (b q) -> p b q", b=NBLK)
                nc.scalar.activation(prv, ltv, AF.Exp)
                ssum = bp.tile([32, NBLK], F32, tag="ssum")
                nc.vector.tensor_reduce(ssum[:], prv, axis=AX.X, op=ALU.add)
                pmin = bp.tile([32, NBLK], F32, tag="pmin")
                nc.vector.tensor_reduce(pmin[:], prv, axis=AX.X, op=ALU.min)
                rs = bp.tile([32, NBLK], F32, tag="rs")
                nc.vector.reciprocal(rs[:], ssum[:])
                den = bp.tile([32, NBLK], F32, tag="den")
                nc.vector.tensor_sub(den[:], ssum[:], pmin[:])
                nc.vector.tensor_mul(den[:], den[:], rs[:])
                nc.vector.tensor_scalar_add(den[:], den[:], 1e-6)
                invd = bp.tile([32, NBLK], F32, tag="invd")
                nc.vector.reciprocal(invd[:], den[:])
                t1 = bp.tile([32, NBLK], F32, tag="t1")
                nc.vector.tensor_mul(t1[:], rs[:], invd[:])
                combt = bp.tile([32, NBLK * 32], BF16, tag="combt")
                cbv = combt[:].rearrange("p (b q) -> p b q", b=NBLK)[:, :, :E]
                mask = bp.tile([32, NBLK * E], F32, tag="mask")
                mkv = mask[:].rearrange("p (b q) -> p b q", b=NBLK)
                pminb = pmin[:].unsqueeze(2).broadcast_to([32, NBLK, E])
                nc.vector.tensor_tensor(mkv, prv, pminb, op=ALU.is_gt)
                nc.vector.tensor_mul(mkv, mkv, prv)
                t1b = t1[:].unsqueeze(2).broadcast_to([32, NBLK, E])
                nc.vector.tensor_tensor(cbv, mkv, t1b, op=ALU.mult)
                combT = bp.tile([32, TC], BF16, tag="combT")
                nc.vector.transpose(combT[:], combt[:])
                return combT

            def expand_block(combT):
                for i2, n2 in enumerate(NT_SIZES):
                    tq = NT_OFF[i2]
                    ex = psH.tile([ER, 512], F32, tag="h", name=f"ex_{i2}")
                    nc.tensor.matmul(ex[:, :n2], bx[:],
                                     combT[:E, tq:tq + n2],
                                     start=True, stop=True)
                    nc.vector.tensor_mul(scaled[:, tq:tq + n2],
                                         acts[:, tq:tq + n2], ex[:, :n2])

            # ---- phase B: base MLP, out accumulates across all 24 j in
            # PSUM; phase A of tiles 1-3 and the softmax/expand interleave
            # into tile 0's j-loop so their inputs arrive under compute ----
            combT = None
            phase_a(0)
            for nt, n in enumerate(NT_SIZES):
                t0 = NT_OFF[nt]
                outp = [psO.tile([128, 512], F32, tag=f"out{m}",
                                 name=f"out{m}_{nt}")
                        for m in range(MC)]
                hsb_prev = None
                for j in range(HC + 1):
                    if j < HC:
                        hps = psH.tile([128, 512], F32, tag="h",
                                       name=f"h_{nt}_{j}")
                        for c in range(DC):
                            nc.tensor.matmul(
                                hps[:, :n],
                                w1v[:, c, j * 128:(j + 1) * 128],
                                xts[nt][:, c * n:(c + 1) * n],
                                start=(c == 0), stop=(c == DC - 1),
                            )
                        hsb = hp.tile([128, 512], BF16, tag="hs",
                                      name=f"hs_{nt}_{j}")
                        nc.scalar.activation(
                            hsb[:, :n], hps[:, :n], AF.Gelu,
                            bias=b1[:, j:j + 1],
                        )
                    if j >= 1:
                        jj = j - 1
                        for m in range(MC):
                            nc.tensor.matmul(
                                outp[m][:, :n],
                                w2v[:, jj, m * 128:(m + 1) * 128],
                                hsb_prev[:, :n],
                                start=(jj == 0), stop=False,
                            )
                    hsb_prev = hsb
                    if nt == 0 and j in (4, 8, 12):
                        phase_a(j // 4)
                        if j == 12:
                            combT = softmax_block()
                    if nt == 0 and j == 19:
                        expand_block(combT)
                # LoRA-up closes each PSUM accumulation group; the PSUM
                # evacuation + bias runs on DVE right behind each chunk
                last = nt == len(NT_SIZES) - 1
                osb = op.tile([128, MC * 512], F32, tag="osb",
                              name=f"osb_{nt}")
                for m in range(MC):
                    nc.tensor.matmul(
                        outp[m][:, :n],
                        wu[:, m * 128:(m + 1) * 128],
                        scaled[:, t0:t0 + n],
                        start=False, stop=True,
                    )
                    if last and m % 2 == 1:
                        nc.scalar.activation(
                            osb[:, m * 512:m * 512 + n], outp[m][:, :n],
                            AF.Identity, bias=b2[:, m:m + 1],
                        )
                    else:
                        nc.vector.tensor_scalar_add(
                            osb[:, m * 512:m * 512 + n], outp[m][:, :n],
                            b2[:, m:m + 1],
                        )
                odv = out_d.rearrange("(m p) t -> p m t", p=128)
                osv = osb[:].rearrange("p (m t) -> p m t", m=MC)
                if last:
                    nc.sync.dma_start(odv[:, :MC // 2, t0:t0 + n],
                                      osv[:, :MC // 2, :n])
                    nc.sync.dma_start(odv[:, MC // 2:, t0:t0 + n],
                                      osv[:, MC // 2:, :n])
                else:
                    nc.sync.dma_start(odv[:, :, t0:t0 + n],
                                      osv[:, :, :n])

    nc.compile()
    return nc


def _pack_rwd(router_w, w_down):
    rw = np.asarray(router_w, np.float32)
    rw_hi = rw.astype(ml_dtypes.bfloat16).astype(np.float32)
    rwd = np.zeros((D, 96), ml_dtypes.bfloat16)
    rwd[:, :E] = rw_hi
    rwd[:, 32:32 + E] = rw - rw_hi
    rwd[:, 64:88] = np.asarray(w_down, np.float32).transpose(1, 0, 2).reshape(D, ER)
    return rwd


def _bf16(a):
    return np.ascontiguousarray(
        np.asarray(a, np.float32).astype(ml_dtypes.bfloat16))


def _prep_inputs(x, w1, b1, w2, b2, router_w, router_b, w_down, w_up):
    x = np.asarray(x, dtype=np.float32)
    xT = x.reshape(T, D).T  # [D, T]
    common = {
        "w1": _bf16(w1),
        "w2": _bf16(w2),
        "wu": _bf16(np.asarray(w_up, np.float32).reshape(ER, D)),
        "b1r": np.ascontiguousarray(
            np.asarray(b1, np.float32).reshape(HC, 128).T),
        "b2r": np.ascontiguousarray(
            np.asarray(b2, np.float32).reshape(MC, 128).T),
        "rwd": np.ascontiguousarray(_pack_rwd(router_w, w_down)),
        "rb": np.ascontiguousarray(
            np.asarray(router_b, np.float32).reshape(E, 1)),
        "bexp": _bf16(np.repeat(np.eye(E, dtype=np.float32), R, axis=1)),
    }
    xT_hi = xT.astype(ml_dtypes.bfloat16)
    xT_lo = (xT - xT_hi.astype(np.float32)).astype(ml_dtypes.bfloat16)
    in_maps = []
    for c in range(NCORES):
        m = dict(common)
        m["xt"] = np.ascontiguousarray(xT_hi[:, c * TC:(c + 1) * TC])
        m["xlo"] = np.ascontiguousarray(xT_lo[:, c * TC:(c + 1) * TC])
        in_maps.append(m)
    return in_maps


def _run(inputs, trace=False):
    if "nc" not in _cache:
        _cache["nc"] = _build()
    nc = _cache["nc"]
    in_maps = _prep_inputs(**inputs)
    res = run_bass_kernel_spmd(nc, in_maps, core_ids=list(range(NCORES)),
                               trace=trace)
    outT = np.concatenate([res.results[c]["outT"] for c in range(NCORES)],
                          axis=1)  # [D, T]
    out = np.ascontiguousarray(outT.T).reshape(B, N, D).astype(np.float32)
    return out, res


def kernel(**inputs):
    return _run(inputs)[0]
